# revision 8
# baseline (speedup 1.0000x reference)
"""Trainium2 Bass kernel for the local-connection GNN message-passing net.

  H[b,i,e] = relu(sum_j A[i,j] * (features[b,j,:] @ weight[i,j,:,:]))
  out[b,i,0] = H[b,i,:] @ pool_weight[:,0]

Strategy (8 NeuronCores, SPMD, no collectives):
  - Shard destination-node axis i into 8 overlapping contiguous slices of 13
    (covers N=100); each core computes its 13 output rows independently.
  - Host prep folds A AND |pool_weight| into the weight slice
    (W''[i,j,d,e] = A[i,j]*|pw[e]|*W[i,j,d,e]) and casts to bf16: halves the
    HBM traffic (10.65 MB/core) and makes the features the SHARED stationary
    matmul operand for all 13 nodes. Columns are permuted so pw>0 columns
    come first; then out = reduce(relu(H''pos)) - reduce(relu(H''neg)),
    since relu(H)*pw == sign(pw)*relu(|pw|*H).
  - Layout W'' as [(j,d) -> 50 chunks of K=128, (il,e) -> 832 free]: per
    chunk just 2 bf16 matmuls (out free 512 + 320, 1 cycle/row) accumulate
    H''[b, il*64+e] over all 50 chunks in two PSUM banks. 100 matmuls total.
  - W'' DMA'd in column granules (small head so the PE starts early, big
    middle granules to minimize ring switches, small tail so the PE drains
    right behind the last transfer); DMA is the roofline (~2.45 TB/s
    aggregate HBM across the 8 cores).
  - Epilogue: relu A-bank on ACT, relu B-bank on DVE (parallel), then two
    free-axis reduces + subtract on DVE; result DMA issued from DVE.
    Output (16,13) f32 per core, host gather.
"""

import numpy as np
from ml_dtypes import bfloat16

B, N, DI, DO = 16, 100, 64, 64
NI = 13  # i-slots per core
STARTS = [0, 13, 26, 39, 52, 61, 74, 87]  # overlapping slices covering 0..99
CH = 50  # (j,d) chunks of 128
FA = 512  # psum A free size (il 0..7)
FB = NI * DO - FA  # psum B free size (il 8..12) = 320
ROW = NI * DO  # 832 weight columns per chunk
GRANULES = [(0, 1), (1, 1), (2, 3), (5, 15), (20, 15), (35, 10), (45, 3),
            (48, 1), (49, 1)]

_cache = {}


def _build_nc(p):
    """p = number of pool_weight entries > 0 (columns are host-permuted so
    those come first within each il block)."""
    import concourse.bacc as bacc
    import concourse.mybir as mybir
    import concourse.tile as tile
    from contextlib import ExitStack

    f32 = mybir.dt.float32
    bf16 = mybir.dt.bfloat16
    nc = bacc.Bacc("TRN2", target_bir_lowering=False, debug=False)

    w_d = nc.dram_tensor("w", [128, CH * ROW], bf16, kind="ExternalInput")
    ft_d = nc.dram_tensor("ft", [128, CH * B], bf16, kind="ExternalInput")
    res_d = nc.dram_tensor("res", [B, NI], f32, kind="ExternalOutput")

    with ExitStack() as ctx:
        tc = ctx.enter_context(tile.TileContext(nc))
        cpool = ctx.enter_context(tc.tile_pool(name="const", bufs=1))
        ppool = ctx.enter_context(tc.tile_pool(name="pp", bufs=1, space="PSUM"))

        ft_sb = cpool.tile([128, CH * B], bf16, tag="ft")
        nc.sync.dma_start(out=ft_sb[:], in_=ft_d[:])

        # PE warm-up: dummy matmuls on an uninitialized scrap tile fill the
        # ~3.5us window between the engine preamble and the first weight
        # granule landing, so the PE reaches full DVFS p-state before the
        # real accumulation starts (cold PE runs at ~1/2 rate and never
        # catches the DMA stream back up).
        scrap_sb = cpool.tile([128, 640], bf16, tag="scrap")
        nc.vector.memset(scrap_sb[:], 0.0)
        scrap_ps = ppool.tile([B, FA], f32, tag="scrap_ps")
        for _ in range(8):
            nc.tensor.matmul(
                scrap_ps[:, :],
                lhsT=scrap_sb[:, 0:B],
                rhs=scrap_sb[:, 128:128 + FA],
                start=True,
                stop=True,
                skip_group_check=True,
            )

        w_tiles = []
        for gi, (c0, ncks) in enumerate(GRANULES):
            wt = cpool.tile([128, ncks * ROW], bf16, tag=f"w{gi}")
            nc.sync.dma_start(out=wt[:], in_=w_d[:, c0 * ROW:(c0 + ncks) * ROW])
            w_tiles.append((c0, ncks, wt))

        psA = ppool.tile([B, FA], f32, tag="psA")
        psB = ppool.tile([B, FB], f32, tag="psB")
        for c0, ncks, wt in w_tiles:
            for k in range(ncks):
                c = c0 + k
                lhs = ft_sb[:, c * B:(c + 1) * B]
                off = k * ROW
                nc.tensor.matmul(
                    psA[:, :],
                    lhsT=lhs,
                    rhs=wt[:, off:off + FA],
                    start=(c == 0),
                    stop=(c == CH - 1),
                    skip_group_check=True,
                )
                nc.tensor.matmul(
                    psB[:, :],
                    lhsT=lhs,
                    rhs=wt[:, off + FA:off + ROW],
                    start=(c == 0),
                    stop=(c == CH - 1),
                    skip_group_check=True,
                )

        # relu the two banks on different engines so they run in parallel;
        # bf16 output halves the DVE reduce time downstream
        r_sb = cpool.tile([B, NI, DO], bf16, tag="r")
        nc.scalar.activation(
            r_sb[:, 0:FA // DO], psA[:, :], mybir.ActivationFunctionType.Relu
        )
        nc.vector.tensor_relu(r_sb[:, FA // DO:NI], psB[:, :])

        # out = sum over pw>0 columns - sum over pw<=0 columns
        res_sb = cpool.tile([B, NI], f32, tag="res")
        if 0 < p < DO:
            s1 = cpool.tile([B, NI], f32, tag="s1")
            s2 = cpool.tile([B, NI], f32, tag="s2")
            nc.vector.tensor_reduce(
                s1[:], r_sb[:, :, 0:p], axis=mybir.AxisListType.X,
                op=mybir.AluOpType.add,
            )
            nc.vector.tensor_reduce(
                s2[:], r_sb[:, :, p:DO], axis=mybir.AxisListType.X,
                op=mybir.AluOpType.add,
            )
            nc.vector.tensor_sub(res_sb[:], s1[:], s2[:])
        else:
            s1 = cpool.tile([B, NI], f32, tag="s1")
            nc.vector.tensor_reduce(
                s1[:], r_sb[:], axis=mybir.AxisListType.X,
                op=mybir.AluOpType.add,
            )
            if p == DO:
                nc.vector.tensor_copy(res_sb[:], s1[:])
            else:
                nc.vector.tensor_scalar_mul(res_sb[:], s1[:], -1.0)
        nc.sync.dma_start(out=res_d[:], in_=res_sb[:])

    nc.compile()
    return nc


def _get_nc(p):
    key = ("nc", p)
    if key not in _cache:
        _cache[key] = _build_nc(p)
    return _cache[key]


def _make_in_maps(features, A, weight, pool_weight):
    features = np.asarray(features, dtype=np.float32)
    A = np.asarray(A, dtype=np.float32)
    weight = np.asarray(weight, dtype=np.float32)
    pw = np.asarray(pool_weight, dtype=np.float32).reshape(DO)

    # permute e columns: pw>0 first; fold |pw| into the weights
    order = np.argsort(pw <= 0, kind="stable")
    p = int((pw > 0).sum())
    wcol = np.abs(pw[order])  # per-e scale after permutation

    # ft[(j,d), b] chunked: ftd[pp, c*16 + b] = features[b, j, d], jd = c*128+pp
    ft = features.transpose(1, 2, 0).reshape(CH * 128, B)
    ftd = np.ascontiguousarray(
        ft.reshape(CH, 128, B).transpose(1, 0, 2).reshape(128, CH * B)
    ).astype(bfloat16)

    in_maps = []
    for c in range(8):
        s = STARTS[c]
        # fold A and |pw| into the weight slice, permute e, lay out as
        # [chunk, p=128, (il,e)]
        wf = weight[s:s + NI][:, :, :, order] * wcol  # (13,100,64,64)
        wf *= A[s:s + NI, :, None, None]
        wf = wf.transpose(1, 2, 0, 3).reshape(CH * 128, ROW)  # [(j,d), (il,e)]
        wd = wf.reshape(CH, 128, ROW).transpose(1, 0, 2).reshape(128, CH * ROW)
        in_maps.append(
            {
                "w": np.ascontiguousarray(wd).astype(bfloat16),
                "ft": ftd,
            }
        )
    return in_maps, p


def _gather(results):
    out = np.zeros((B, N), np.float32)
    for c in range(8):
        r = np.asarray(results[c]["res"], dtype=np.float32)  # (16, 13)
        out[:, STARTS[c]:STARTS[c] + NI] = r
    return out[:, :, None]


def run(features, A, weight, pool_weight, trace=False, **trace_kwargs):
    from concourse.bass_utils import run_bass_kernel_spmd

    in_maps, p = _make_in_maps(features, A, weight, pool_weight)
    nc = _get_nc(p)
    br = run_bass_kernel_spmd(
        nc, in_maps, core_ids=list(range(8)), trace=trace, **trace_kwargs
    )
    return _gather(br.results), br


def kernel(features, A, weight, pool_weight):
    out, _ = run(features, A, weight, pool_weight)
    return out


# revision 9
# speedup vs baseline: 1.1340x; 1.1340x over previous
"""Trainium2 Bass kernel for the local-connection GNN message-passing net.

  H[b,i,e] = relu(sum_j A[i,j] * (features[b,j,:] @ weight[i,j,:,:]))
  out[b,i,0] = H[b,i,:] @ pool_weight[:,0]

Strategy (8 NeuronCores, SPMD, no collectives):
  - Shard destination-node axis i into 8 overlapping contiguous slices of 13
    (covers N=100); each core computes its 13 output rows independently.
  - Host prep folds A AND |pool_weight| into the weight slice
    (W''[i,j,d,e] = A[i,j]*|pw[e]|*W[i,j,d,e]) and casts to bf16: halves the
    HBM traffic (10.65 MB/core) and makes the features the SHARED stationary
    matmul operand for all 13 nodes. Columns are permuted so pw>0 columns
    come first; then out = reduce(relu(H''pos)) - reduce(relu(H''neg)),
    since relu(H)*pw == sign(pw)*relu(|pw|*H).
  - Layout W'' as [(j,d) -> 50 chunks of K=128, (il,e) -> 832 free]: per
    chunk just 2 bf16 matmuls (out free 512 + 320, 1 cycle/row) accumulate
    H''[b, il*64+e] over all 50 chunks in two PSUM banks. 100 matmuls total.
  - The transposed features ride in front of weight chunk 0 in one DRAM
    tensor, so granule 0's DMA arms the first matmul by itself. Granules:
    small head so the PE starts early, big middle granules (one 25-30KB
    descriptor per partition) for max DMA rate, small tail so the PE drains
    right behind the last transfer. Stream runs at ~390 GB/s/core.
  - Keep total work minimal: extra engine activity triggers chip-level DVFS
    throttling that slows PE and DMA together (measured +13us from 8 scrap
    warm-up matmuls).
  - Epilogue: relu A-bank on ACT, relu B-bank on DVE (parallel, bf16 out),
    then two free-axis reduces + subtract on DVE; result DMA from the warm
    Sync queue. Output (16,13) f32 per core, host gather.
"""

import numpy as np
from ml_dtypes import bfloat16

B, N, DI, DO = 16, 100, 64, 64
NI = 13  # i-slots per core
STARTS = [0, 13, 26, 39, 52, 61, 74, 87]  # overlapping slices covering 0..99
CH = 50  # (j,d) chunks of 128
FA = 512  # psum A free size (il 0..7)
FB = NI * DO - FA  # psum B free size (il 8..12) = 320
ROW = NI * DO  # 832 weight columns per chunk
FT = CH * B  # 800 columns of transposed features ahead of the chunks
# (start_chunk, n_chunks) DMA granules; granule 0 also carries the features
GRANULES = [(0, 1), (1, 1), (2, 3), (5, 15), (20, 15), (35, 10), (45, 4),
            (49, 1)]

_cache = {}


def _build_nc(p):
    """p = number of pool_weight entries > 0 (columns are host-permuted so
    those come first within each il block)."""
    import concourse.bacc as bacc
    import concourse.mybir as mybir
    import concourse.tile as tile
    from contextlib import ExitStack

    f32 = mybir.dt.float32
    bf16 = mybir.dt.bfloat16
    nc = bacc.Bacc("TRN2", target_bir_lowering=False, debug=False)

    w_d = nc.dram_tensor("w", [128, FT + CH * ROW], bf16, kind="ExternalInput")
    res_d = nc.dram_tensor("res", [B, NI], f32, kind="ExternalOutput")

    with ExitStack() as ctx:
        tc = ctx.enter_context(tile.TileContext(nc))
        cpool = ctx.enter_context(tc.tile_pool(name="const", bufs=1))
        ppool = ctx.enter_context(tc.tile_pool(name="pp", bufs=1, space="PSUM"))

        w_tiles = []
        for gi, (c0, ncks) in enumerate(GRANULES):
            lo = c0 * ROW + (0 if gi == 0 else FT)
            hi = (c0 + ncks) * ROW + FT
            wt = cpool.tile([128, hi - lo], bf16, tag=f"w{gi}")
            nc.sync.dma_start(out=wt[:], in_=w_d[:, lo:hi])
            w_tiles.append((c0, ncks, wt))
        ft_sb = w_tiles[0][2]  # features live at the head of granule 0

        psA = ppool.tile([B, FA], f32, tag="psA")
        psB = ppool.tile([B, FB], f32, tag="psB")
        for c0, ncks, wt in w_tiles:
            for k in range(ncks):
                c = c0 + k
                lhs = ft_sb[:, c * B:(c + 1) * B]
                off = k * ROW + (FT if c0 == 0 else 0)
                nc.tensor.matmul(
                    psA[:, :],
                    lhsT=lhs,
                    rhs=wt[:, off:off + FA],
                    start=(c == 0),
                    stop=(c == CH - 1),
                    skip_group_check=True,
                )
                nc.tensor.matmul(
                    psB[:, :],
                    lhsT=lhs,
                    rhs=wt[:, off + FA:off + ROW],
                    start=(c == 0),
                    stop=(c == CH - 1),
                    skip_group_check=True,
                )

        # relu the two banks on different engines so they run in parallel;
        # bf16 output halves the DVE reduce time downstream
        r_sb = cpool.tile([B, NI, DO], bf16, tag="r")
        nc.scalar.activation(
            r_sb[:, 0:FA // DO], psA[:, :], mybir.ActivationFunctionType.Relu
        )
        nc.vector.tensor_relu(r_sb[:, FA // DO:NI], psB[:, :])

        # out = sum over pw>0 columns - sum over pw<=0 columns
        res_sb = cpool.tile([B, NI], f32, tag="res")
        if 0 < p < DO:
            s1 = cpool.tile([B, NI], f32, tag="s1")
            s2 = cpool.tile([B, NI], f32, tag="s2")
            nc.vector.tensor_reduce(
                s1[:], r_sb[:, :, 0:p], axis=mybir.AxisListType.X,
                op=mybir.AluOpType.add,
            )
            nc.vector.tensor_reduce(
                s2[:], r_sb[:, :, p:DO], axis=mybir.AxisListType.X,
                op=mybir.AluOpType.add,
            )
            nc.vector.tensor_sub(res_sb[:], s1[:], s2[:])
        else:
            s1 = cpool.tile([B, NI], f32, tag="s1")
            nc.vector.tensor_reduce(
                s1[:], r_sb[:], axis=mybir.AxisListType.X,
                op=mybir.AluOpType.add,
            )
            if p == DO:
                nc.vector.tensor_copy(res_sb[:], s1[:])
            else:
                nc.vector.tensor_scalar_mul(res_sb[:], s1[:], -1.0)
        nc.sync.dma_start(out=res_d[:], in_=res_sb[:])

    nc.compile()
    return nc


def _get_nc(p):
    key = ("nc", p)
    if key not in _cache:
        _cache[key] = _build_nc(p)
    return _cache[key]


def _make_in_maps(features, A, weight, pool_weight):
    features = np.asarray(features, dtype=np.float32)
    A = np.asarray(A, dtype=np.float32)
    weight = np.asarray(weight, dtype=np.float32)
    pw = np.asarray(pool_weight, dtype=np.float32).reshape(DO)

    # permute e columns: pw>0 first; fold |pw| into the weights
    order = np.argsort(pw <= 0, kind="stable")
    p = int((pw > 0).sum())
    wcol = np.abs(pw[order])  # per-e scale after permutation

    # ft[(j,d), b] chunked: ftd[pp, c*16 + b] = features[b, j, d], jd = c*128+pp
    ft = features.transpose(1, 2, 0).reshape(CH * 128, B)
    ftd = np.ascontiguousarray(
        ft.reshape(CH, 128, B).transpose(1, 0, 2).reshape(128, FT)
    )

    in_maps = []
    for c in range(8):
        s = STARTS[c]
        # fold A and |pw| into the weight slice, permute e, lay out as
        # [chunk, p=128, (il,e)]; features ride ahead of chunk 0
        wf = weight[s:s + NI][:, :, :, order] * wcol  # (13,100,64,64)
        wf *= A[s:s + NI, :, None, None]
        wf = wf.transpose(1, 2, 0, 3).reshape(CH * 128, ROW)  # [(j,d), (il,e)]
        wd = wf.reshape(CH, 128, ROW).transpose(1, 0, 2).reshape(128, CH * ROW)
        wm = np.concatenate([ftd, wd], axis=1)
        in_maps.append({"w": wm.astype(bfloat16)})
    return in_maps, p


def _gather(results):
    out = np.zeros((B, N), np.float32)
    for c in range(8):
        r = np.asarray(results[c]["res"], dtype=np.float32)  # (16, 13)
        out[:, STARTS[c]:STARTS[c] + NI] = r
    return out[:, :, None]


def run(features, A, weight, pool_weight, trace=False, **trace_kwargs):
    from concourse.bass_utils import run_bass_kernel_spmd

    in_maps, p = _make_in_maps(features, A, weight, pool_weight)
    nc = _get_nc(p)
    br = run_bass_kernel_spmd(
        nc, in_maps, core_ids=list(range(8)), trace=trace, **trace_kwargs
    )
    return _gather(br.results), br


def kernel(features, A, weight, pool_weight):
    out, _ = run(features, A, weight, pool_weight)
    return out
